# revision 62
# baseline (speedup 1.0000x reference)
"""Single-head attention (B=4, S=2048, E=1024, H=64) on 8 trn2 NeuronCores.

Sharding: core c -> batch b = c//2, query-half h = c%2. Each core projects
q/k/v for its own 1024 tokens (bf16), pair-exchanges [k^T | v-transposed]
via a 2-core AllReduce (peer = sum - mine), and runs softmax(q k^T / 8) v
for its 1024 queries over all 2048 keys.

Primary layout "p" (see _emit_p): the per-rep bodies are software-pipelined
at EMISSION level -- engines execute their queues in program order, so
block i interleaves rep i-2's attention with rep i's projections; the PE
fills exp(ACT)-bound gaps with the next rep's projection matmuls, x DMAs
are prefetched one block early, and the collective gets ~2 blocks of
latency cover. All tiles come from ring-buffered pools (PSUM: pao 1 +
psS 4 + psO 2 + ptp 1 = 8 banks).

Per rep: projections stream x^T once through [Wk|Wv] (k evicted with bias
into a partition-split dup layout for row-packed score matmuls) plus 4
col-packed q passes ([Wq_even|Wq_odd], partials summed on DVE, result
duplicated across partition halves). v^T is PE-transposed [128,128]-wise
into the exchange payload so the peer's v arrives PV-ready. Attention per
(query-block, key-tile-pair) group: two row-packed S^T matmuls (contraction
64: array rows 0:63 / 64:127 concurrently), exp on ACT with a one-group
S-lookahead in PE program order, PV accumulation with a ones column
carrying softmax denominators, then PE-transpose + reciprocal-scale + bv.
"""

import numpy as np
from contextlib import ExitStack

import concourse.bacc as bacc
import concourse.tile as tile
from concourse import mybir
from concourse.masks import make_identity

B, S, E, H = 4, 2048, 1024, 64
NCORES = 8
HALF = S // 2  # queries per core
ET = E // 128  # e-tiles
KT = S // 128  # k-tiles
F32 = mybir.dt.float32

SCALE = 0.125  # 1/sqrt(H)

# PV matmul in fp8e4 with DoubleRow (2 key-tiles per pass). Measured: no
# throughput win (DoubleRow disables FWL; the heavier LDWEIGHTS cancels the
# matmul savings at M=65) and it costs accuracy (1.8e-2 vs 5.5e-3), so OFF.
PV_FP8 = False
FP8 = mybir.dt.float8e4

# Software-pipeline depth: attention of rep i runs interleaved with the
# front of rep i+PIPE, giving the pair-collective PIPE-1 full blocks of
# latency cover. Ring sizes below must keep PIPE+1 reps' exchange state.
# Depth 3 measured no better than 2 (the collective cost is serialization,
# not exposed latency), so stay at 2.
PIPE = 2


def _emit_rep(nc, tc, DT, dram, consts, r, upto="full", layout="dup"):
    """One full iteration of the kernel body (DMA + all phases)."""
    xt, out = dram["xt"], dram["out"]
    wvq_sb, wk_sb, wvk_sb = consts["wvq"], consts["wk"], consts["wvk"]
    b0q_sb, b0k_sb, bvb_sb = consts["b0q"], consts["b0k"], consts["bvb"]
    ident, ident_r = consts["ident"], consts["ident_r"]

    with ExitStack() as ctx:
        persist = ctx.enter_context(tc.tile_pool(name=f"persist{r}", bufs=1))

        # --- x^T (e-permuted: partition p holds e-rows 8p..8p+7, one
        # 1024-col s-segment per e).  4 x 1MB DMA chunks per half. ---
        xt_own = persist.tile([128, ET * HALF], DT, tag="xt_own",
                              name=f"xt_own_{r}")
        xt_oth = persist.tile([128, ET * HALF], DT, tag="xt_oth",
                              name=f"xt_oth_{r}")
        CH = ET * HALF // 4  # 2048 elems per partition per chunk
        _engs = [nc.sync, nc.scalar, nc.gpsimd]
        for c in range(4):
            _engs[c % 3].dma_start(
                out=xt_own[:, c * CH:(c + 1) * CH], in_=xt[0, :, c, :])
        for c in range(4):
            _engs[(c + 1) % 3].dma_start(
                out=xt_oth[:, c * CH:(c + 1) * CH], in_=xt[1, :, c, :])

        def xo_slice(t, et, blk):
            o = et * HALF + blk * 512
            return t[:, o:o + 512]

        # --- projection outputs ---
        # vq_own rows 0:64 = v^T_own, rows 64:128 = q^T_own
        vq_own = persist.tile([128, HALF], DT, tag="vq_own", name=f"vq_own_{r}")
        # k_own rows 64:128 = k^T of own half
        k_own = persist.tile([128, HALF], DT, tag="k_own", name=f"k_own_{r}")
        # vk_oth rows 0:64 = v^T_oth, rows 64:128 = k^T_oth
        vk_oth = persist.tile([128, HALF], DT, tag="vk_oth", name=f"vk_oth_{r}")
        v_sb = [
            persist.tile([128, H + 1], DT, tag=f"v{kt}", name=f"v{kt}_{r}")
            for kt in range(KT)
        ]
        f_all = persist.tile(
            [128, 2 * HALF // 256, H], F32, tag="f_all", name=f"f_all_{r}"
        )

        if upto == "dma":
            # DMA only + one anchor matmul per half so DCE keeps the loads
            with tc.tile_pool(name=f"pad{r}", bufs=1, space="PSUM") as pad:
                psd = pad.tile([128, 512], F32, tag="d", name=f"psd_{r}")
                for et in range(ET):
                    nc.tensor.matmul(psd, wvq_sb[:, et], xo_slice(xt_own, et, 0),
                                     start=(et == 0), stop=(et == ET - 1))
                for et in range(ET):
                    nc.tensor.matmul(psd, wvq_sb[:, et], xo_slice(xt_oth, et, 0),
                                     start=(et == 0), stop=(et == ET - 1))
                nc.vector.tensor_scalar_add(vq_own[:, 0:512], psd, b0q_sb)
                nc.vector.memset(f_all, 0.0)
                nc.sync.dma_start(
                    out=out.rearrange("(t p) d -> p t d", p=128),
                    in_=f_all)
            return

        if layout == "cc2":
            # Like cc, but the pair exchange is AllReduce(add) + local
            # subtract (no rank-dependent slot selection), and attention over
            # the core's own 8 k-tiles starts before the exchange completes
            # (softmax accumulation is key-order invariant).
            kv_mine = persist.tile([128, HALF], DT, tag="kv_mine",
                                   name=f"kv_mine_{r}")
            kv_peer = persist.tile([128, HALF], DT, tag="kv_peer",
                                   name=f"kv_peer_{r}")
            q_own = persist.tile([H, HALF], DT, tag="q_own", name=f"q_own_{r}")
            with tc.tile_pool(name=f"pao{r}", bufs=1, space="PSUM") as pao:
                for blk in range(2):
                    ps_kv = pao.tile([128, 512], F32, tag=f"akv{blk}",
                                     name=f"akv{blk}_{r}")
                    ps_q = pao.tile([H, 512], F32, tag=f"aq{blk}",
                                    name=f"aq{blk}_{r}")
                    for et in range(ET):
                        st = dict(start=(et == 0), stop=(et == ET - 1))
                        xo = xo_slice(xt_own, et, blk)
                        nc.tensor.matmul(ps_kv, consts["wkv2"][:, et], xo, **st)
                        nc.tensor.matmul(ps_q, consts["wq"][:, et], xo, **st)
                    c0, c1 = blk * 512, (blk + 1) * 512
                    nc.vector.tensor_scalar_add(
                        kv_mine[:, c0:c1], ps_kv, consts["bkv"])
                    nc.vector.tensor_scalar_add(
                        q_own[:, c0:c1], ps_q, consts["bq64"])
            with tc.tile_pool(name=f"dr{r}", bufs=1, space="DRAM") as drp:
                kv_local = drp.tile([128, HALF], DT, tag="kvl",
                                    name=f"kv_local_{r}")
                kv_sum = drp.tile([128, HALF], DT, tag="kvs",
                                  name=f"kv_sum_{r}")
                nc.sync.dma_start(out=kv_local[:], in_=kv_mine)
                nc.gpsimd.collective_compute(
                    "AllReduce",
                    mybir.AluOpType.add,
                    replica_groups=[[0, 1], [2, 3], [4, 5], [6, 7]],
                    ins=[kv_local.opt()],
                    outs=[kv_sum.opt()],
                )
                kv_sum_sb = persist.tile([128, HALF], DT, tag="kv_sum_sb",
                                         name=f"kv_sum_sb_{r}")
                nc.scalar.dma_start(out=kv_sum_sb, in_=kv_sum[:])
                nc.vector.tensor_sub(kv_peer, kv_sum_sb, kv_mine)

            psS = ctx.enter_context(
                tc.tile_pool(name=f"psS{r}", bufs=2, space="PSUM"))
            psO = ctx.enter_context(
                tc.tile_pool(name=f"psO{r}", bufs=2, space="PSUM"))
            pt = ctx.enter_context(
                tc.tile_pool(name=f"pt{r}", bufs=2, space="PSUM"))
            ppool = ctx.enter_context(tc.tile_pool(name=f"pp{r}", bufs=3))
            opool = ctx.enter_context(tc.tile_pool(name=f"op{r}", bufs=4))

            def kvsrc(kt):
                t = kv_mine if kt < 8 else kv_peer
                kk = kt % 8
                return t, kk * 128, (kk + 1) * 128

            def v_transpose2(kt):
                t, a, b = kvsrc(kt)
                pst = pt.tile([128, H + 1], DT, tag="tr", name=f"pst{kt}_{r}")
                nc.tensor.transpose(
                    pst[:, 0:H], t[64:128, a:b], ident_r[64:128, 64:128])
                nc.vector.tensor_copy(v_sb[kt][:, 0:H], pst[:, 0:H])
                ones_col = v_sb[kt][:, H:H + 1]
                nc.vector.memset(
                    ones_col.bitcast(F32) if DT == mybir.dt.float32r
                    else ones_col, 1.0)

            for kt in range(8):
                v_transpose2(kt)

            for qb in range(HALF // 512):
                q_ap = q_own[:, qb * 512:(qb + 1) * 512]
                o_ps = psO.tile([H + 1, 512], F32, tag="o", name=f"o_ps{qb}_{r}")
                for g in range(KT // 2):
                    if qb == 0 and g == 4:
                        for kt in range(8, KT):
                            v_transpose2(kt)
                    s_ps = psS.tile([128, 2, 512], F32, tag="s",
                                    name=f"s_ps{qb}_{g}_{r}")
                    for j in range(2):
                        kt = g * 2 + j
                        t, a, b = kvsrc(kt)
                        nc.tensor.matmul(s_ps[:, j], t[0:64, a:b], q_ap,
                                         start=True, stop=True)
                    p_sb = ppool.tile([128, 2, 512], DT, tag="p",
                                      name=f"p_sb{qb}_{g}_{r}")
                    nc.scalar.activation(
                        p_sb, s_ps, mybir.ActivationFunctionType.Exp,
                        scale=SCALE)
                    for j in range(2):
                        kt = g * 2 + j
                        nc.tensor.matmul(
                            o_ps, v_sb[kt], p_sb[:, j],
                            start=(kt == 0), stop=(kt == KT - 1),
                        )
                for sub in range(4):
                    o_t = opool.tile([H + 1, 128], F32, tag="ot",
                                     name=f"ot{qb}{sub}_{r}")
                    nc.vector.tensor_copy(o_t, o_ps[:, sub * 128:(sub + 1) * 128])
                    ps2 = pt.tile([128, H + 1], F32, tag="tr",
                                  name=f"ps2_{qb}{sub}_{r}")
                    nc.tensor.transpose(ps2, o_t, ident[0:H + 1, 0:H + 1])
                    r_t = opool.tile([128, 1], F32, tag="rt",
                                     name=f"rt{qb}{sub}_{r}")
                    nc.vector.reciprocal(r_t, ps2[:, H:H + 1])
                    fa = f_all[:, qb * 4 + sub, :]
                    nc.vector.tensor_scalar_mul(fa, ps2[:, 0:H], r_t)
                    nc.vector.tensor_add(fa, fa, bvb_sb)

            nc.sync.dma_start(
                out=out.rearrange("(t p) d -> p t d", p=128), in_=f_all)
            return

        if layout == "cc":
            # Sequence-parallel projections: each core projects only its own
            # half (k^T rows 0:64 + bk, v^T rows 64:128, q^T separately),
            # then pairwise-AllGathers [k^T; v^T] so both cores hold the
            # full-sequence k/v.
            kv_mine = persist.tile([128, HALF], DT, tag="kv_mine",
                                   name=f"kv_mine_{r}")
            q_own = persist.tile([H, HALF], DT, tag="q_own", name=f"q_own_{r}")
            kv_all = persist.tile([128, 2, HALF], DT, tag="kv_all",
                                  name=f"kv_all_{r}")
            with tc.tile_pool(name=f"pao{r}", bufs=1, space="PSUM") as pao:
                for blk in range(2):
                    ps_kv = pao.tile([128, 512], F32, tag=f"akv{blk}",
                                     name=f"akv{blk}_{r}")
                    ps_q = pao.tile([H, 512], F32, tag=f"aq{blk}",
                                    name=f"aq{blk}_{r}")
                    for et in range(ET):
                        st = dict(start=(et == 0), stop=(et == ET - 1))
                        xo = xo_slice(xt_own, et, blk)
                        nc.tensor.matmul(ps_kv, consts["wkv2"][:, et], xo, **st)
                        nc.tensor.matmul(ps_q, consts["wq"][:, et], xo, **st)
                    c0, c1 = blk * 512, (blk + 1) * 512
                    nc.vector.tensor_scalar_add(
                        kv_mine[:, c0:c1], ps_kv, consts["bkv"])
                    nc.vector.tensor_scalar_add(
                        q_own[:, c0:c1], ps_q, consts["bq64"])
            with tc.tile_pool(name=f"dr{r}", bufs=1, space="DRAM") as drp:
                kv_local = drp.tile([128, HALF], DT, tag="kvl",
                                    name=f"kv_local_{r}")
                kv_pair = drp.tile([2, 128, HALF], DT, tag="kvp",
                                   name=f"kv_pair_{r}")
                nc.sync.dma_start(out=kv_local[:], in_=kv_mine)
                nc.gpsimd.collective_compute(
                    "AllGather",
                    mybir.AluOpType.bypass,
                    replica_groups=[[0, 1], [2, 3], [4, 5], [6, 7]],
                    ins=[kv_local.opt()],
                    outs=[kv_pair.opt()],
                )
                for s in range(2):
                    nc.sync.dma_start(out=kv_all[:, s, :], in_=kv_pair[s])

            psS = ctx.enter_context(
                tc.tile_pool(name=f"psS{r}", bufs=2, space="PSUM"))
            psO = ctx.enter_context(
                tc.tile_pool(name=f"psO{r}", bufs=1, space="PSUM"))
            pt = ctx.enter_context(
                tc.tile_pool(name=f"pt{r}", bufs=2, space="PSUM"))
            ppool = ctx.enter_context(tc.tile_pool(name=f"pp{r}", bufs=3))
            opool = ctx.enter_context(tc.tile_pool(name=f"op{r}", bufs=4))

            for kt in range(KT):
                slot, kk = kt // 8, kt % 8
                srcv = kv_all[64:128, slot, kk * 128:(kk + 1) * 128]
                pst = pt.tile([128, H + 1], DT, tag="tr", name=f"pst{kt}_{r}")
                nc.tensor.transpose(
                    pst[:, 0:H], srcv, ident_r[64:128, 64:128])
                nc.vector.tensor_copy(v_sb[kt][:, 0:H], pst[:, 0:H])
                ones_col = v_sb[kt][:, H:H + 1]
                nc.vector.memset(
                    ones_col.bitcast(F32) if DT == mybir.dt.float32r else ones_col,
                    1.0)

            for qb in range(HALF // 512):
                q_ap = q_own[:, qb * 512:(qb + 1) * 512]
                o_ps = psO.tile([H + 1, 512], F32, tag="o", name=f"o_ps{qb}_{r}")
                for g in range(KT // 2):
                    s_ps = psS.tile([128, 2, 512], F32, tag="s",
                                    name=f"s_ps{qb}_{g}_{r}")
                    for j in range(2):
                        kt = g * 2 + j
                        slot, kk = kt // 8, kt % 8
                        kl = kv_all[0:64, slot, kk * 128:(kk + 1) * 128]
                        nc.tensor.matmul(s_ps[:, j], kl, q_ap,
                                         start=True, stop=True)
                    p_sb = ppool.tile([128, 2, 512], DT, tag="p",
                                      name=f"p_sb{qb}_{g}_{r}")
                    nc.scalar.activation(
                        p_sb, s_ps, mybir.ActivationFunctionType.Exp,
                        scale=SCALE)
                    for j in range(2):
                        kt = g * 2 + j
                        nc.tensor.matmul(
                            o_ps, v_sb[kt], p_sb[:, j],
                            start=(kt == 0), stop=(kt == KT - 1),
                        )
                for sub in range(4):
                    o_t = opool.tile([H + 1, 128], F32, tag="ot",
                                     name=f"ot{qb}{sub}_{r}")
                    nc.vector.tensor_copy(o_t, o_ps[:, sub * 128:(sub + 1) * 128])
                    ps2 = pt.tile([128, H + 1], F32, tag="tr",
                                  name=f"ps2_{qb}{sub}_{r}")
                    nc.tensor.transpose(ps2, o_t, ident[0:H + 1, 0:H + 1])
                    r_t = opool.tile([128, 1], F32, tag="rt",
                                     name=f"rt{qb}{sub}_{r}")
                    nc.vector.reciprocal(r_t, ps2[:, H:H + 1])
                    fa = f_all[:, qb * 4 + sub, :]
                    nc.vector.tensor_scalar_mul(fa, ps2[:, 0:H], r_t)
                    nc.vector.tensor_add(fa, fa, bvb_sb)

            nc.sync.dma_start(
                out=out.rearrange("(t p) d -> p t d", p=128), in_=f_all)
            return

        # ---------- Phase A (own half): q^T, k^T_own, v^T_own ----------
        with tc.tile_pool(name=f"pao{r}", bufs=1, space="PSUM") as pao:
            ps_vq = [
                pao.tile([128, 512], F32, tag=f"avq{i}", name=f"avq{i}_{r}")
                for i in range(2)
            ]
            ps_k = [
                pao.tile([128, 512], F32, tag=f"ak{i}", name=f"ak{i}_{r}")
                for i in range(2)
            ]
            for et in range(ET):
                st = dict(start=(et == 0), stop=(et == ET - 1))
                for blk in range(2):
                    xo = xo_slice(xt_own, et, blk)
                    nc.tensor.matmul(ps_vq[blk], wvq_sb[:, et], xo, **st)
                    nc.tensor.matmul(ps_k[blk], wk_sb[:, et], xo, **st)
            for blk in range(2):
                c0, c1 = blk * 512, (blk + 1) * 512
                nc.vector.tensor_scalar_add(vq_own[:, c0:c1], ps_vq[blk], b0q_sb)
                nc.vector.tensor_scalar_add(
                    k_own[64:128, c0:c1], ps_k[blk][64:128, :], b0k_sb[64:128, :]
                )

        # Remaining PSUM budget (8 banks): psS 4 + psO 1 + pt 2 + pa2 1
        psS = ctx.enter_context(tc.tile_pool(name=f"psS{r}", bufs=2, space="PSUM"))
        psO = ctx.enter_context(tc.tile_pool(name=f"psO{r}", bufs=1, space="PSUM"))
        pt = ctx.enter_context(tc.tile_pool(name=f"pt{r}", bufs=2, space="PSUM"))
        pa2 = ctx.enter_context(tc.tile_pool(name=f"pa2{r}", bufs=1, space="PSUM"))
        ppool = ctx.enter_context(tc.tile_pool(name=f"pp{r}", bufs=3))
        opool = ctx.enter_context(tc.tile_pool(name=f"op{r}", bufs=4))

        def v_transpose(kt):
            src = (
                vq_own[0:64, kt * 128:(kt + 1) * 128]
                if kt < 8
                else vk_oth[0:64, (kt - 8) * 128:(kt - 7) * 128]
            )
            pst = pt.tile([128, H + 1], DT, tag="tr", name=f"pst{kt}_{r}")
            nc.tensor.transpose(pst[:, 0:H], src, ident_r[0:64, 0:64])
            nc.vector.tensor_copy(v_sb[kt][:, 0:H], pst[:, 0:H])
            ones_col = v_sb[kt][:, H:H + 1]
            nc.vector.memset(
                ones_col.bitcast(F32) if DT == mybir.dt.float32r else ones_col,
                1.0)

        # ---------- Phase A2 (own half v tiles) ----------
        for kt in range(8):
            v_transpose(kt)

        # ---------- Phase A (other half): k^T_oth, v^T_oth ----------
        for blk in range(2):
            ps_vk = pa2.tile([128, 512], F32, tag="avk", name=f"avk{blk}_{r}")
            for et in range(ET):
                xf = xo_slice(xt_oth, et, blk)
                nc.tensor.matmul(
                    ps_vk, wvk_sb[:, et], xf,
                    start=(et == 0), stop=(et == ET - 1),
                )
            nc.vector.tensor_scalar_add(
                vk_oth[:, blk * 512:(blk + 1) * 512], ps_vk, b0k_sb
            )
        for kt in range(8, KT):
            v_transpose(kt)

        if upto == "proj":
            nc.vector.memset(f_all, 0.0)
            nc.sync.dma_start(
                out=out.rearrange("(t p) d -> p t d", p=128), in_=f_all)
            return

        # ---------- Phase B + C: attention per query block ----------
        for qb in range(HALF // 512):
            q_ap = vq_own[64:128, qb * 512:(qb + 1) * 512]
            o_ps = psO.tile([H + 1, 512], F32, tag="o", name=f"o_ps{qb}_{r}")
            for g in range(KT // 2):
                s_ps = psS.tile([128, 2, 512], F32, tag="s", name=f"s_ps{qb}_{g}_{r}")
                for j in range(2):
                    kt = g * 2 + j
                    kl = (
                        k_own[64:128, kt * 128:(kt + 1) * 128]
                        if kt < 8
                        else vk_oth[64:128, (kt - 8) * 128:(kt - 7) * 128]
                    )
                    nc.tensor.matmul(s_ps[:, j], kl, q_ap, start=True, stop=True)
                p_sb = ppool.tile([128, 2, 512], DT, tag="p", name=f"p_sb{qb}_{g}_{r}")
                nc.scalar.activation(
                    p_sb, s_ps, mybir.ActivationFunctionType.Exp, scale=SCALE
                )
                for j in range(2):
                    kt = g * 2 + j
                    nc.tensor.matmul(
                        o_ps, v_sb[kt], p_sb[:, j],
                        start=(kt == 0), stop=(kt == KT - 1),
                    )
            if upto == "attn":
                nc.vector.tensor_copy(f_all[0:65, qb * 4, :], o_ps[:, 0:64])
                continue
            for sub in range(4):
                o_t = opool.tile([H + 1, 128], F32, tag="ot", name=f"ot{qb}{sub}_{r}")
                nc.vector.tensor_copy(o_t, o_ps[:, sub * 128:(sub + 1) * 128])
                ps2 = pt.tile([128, H + 1], F32, tag="tr", name=f"ps2_{qb}{sub}_{r}")
                nc.tensor.transpose(ps2, o_t, ident[0:H + 1, 0:H + 1])
                r_t = opool.tile([128, 1], F32, tag="rt", name=f"rt{qb}{sub}_{r}")
                nc.vector.reciprocal(r_t, ps2[:, H:H + 1])
                fa = f_all[:, qb * 4 + sub, :]
                nc.vector.tensor_scalar_mul(fa, ps2[:, 0:H], r_t)
                nc.vector.tensor_add(fa, fa, bvb_sb)

        nc.sync.dma_start(out=out.rearrange("(t p) d -> p t d", p=128), in_=f_all)


def _front_gen_p(nc, tc, DT, dram, consts, r, st, pools):
    """Pipelined-cc front half of rep r: DMA x-own, project [k|k] and [v|q],
    evict into the exchange tile + duplicated q, launch the pair AllReduce.

    Yields between micro-steps so the driver can interleave this emission
    with attention of rep r-2 (fills PE gaps while ACT runs the exps).
    """
    xt = dram["xt"]
    sb, dr, pao = pools["sb"], pools["dr"], pools["pao"]
    local = st.get("local")  # pd mode: project peer half locally
    xt_own = sb.tile([128, ET * HALF], DT, tag="xt", name=f"pxt_{r}", bufs=3)
    CH = ET * HALF // 4
    for c in range(4):
        nc.sync.dma_start(out=xt_own[:, c * CH:(c + 1) * CH], in_=xt[0, :, c, :])
    if local:
        xt_oth = sb.tile([128, ET * HALF], DT, tag="xto", name=f"pxto_{r}",
                         bufs=3)
        for c in range(4):
            nc.sync.dma_start(
                out=xt_oth[:, c * CH:(c + 1) * CH], in_=xt[1, :, c, :])
    # ex_sb cols 0:512 = k^T (rows 0:64 = block-0 tiles 0..3, rows 64:128 =
    # block-1 tiles 4..7), cols 512:1024 = v^T in the same block split.
    ex_sb = sb.tile([128, 1024], DT, tag="ex", name=f"pex_{r}",
                    bufs=PIPE + 1)
    qq = sb.tile([128, HALF], DT, tag="qq", name=f"pqq_{r}",
                 bufs=PIPE + 1)
    st.update(ex_sb=ex_sb, qq=qq)
    yield
    wkv2, wqp2 = consts["wkv2"], consts["wqp2"]
    bkk, b0q = consts["bkk"], consts["b0q"]
    ptp, ident_r = pools["ptp"], consts["ident_r"]
    # v^T staging tile; transposed into the exchange payload below so the
    # peer's v arrives PV-ready and the attention phase does no transposes.
    vtt = sb.tile([128, 512], DT, tag="vtt", name=f"pvtt_{r}", bufs=2)
    for b in range(2):
        c0, c1 = b * 512, (b + 1) * 512
        h0, h1 = b * 64, (b + 1) * 64
        psA = pao.tile([128, 4, 128], F32, tag="a", name=f"ppsA{b}_{r}")
        for et in range(ET):
            xo = xt_own[:, et * HALF + c0:et * HALF + c1]
            nc.tensor.matmul(psA, wkv2[:, et], xo,
                             start=(et == 0), stop=(et == ET - 1))
            if et % 2 == 1:
                yield
        # [k|v] psum: k^T from rows 0:64 (block b -> partition half b of the
        # kk region), v^T from rows 64:128 (staged, transposed below).
        nc.vector.tensor_scalar_add(
            ex_sb[h0:h1, 0:512], psA[0:64, :, :], bkk[0:64])
        nc.vector.tensor_copy(vtt[h0:h1, :], psA[64:128, :, :])
        yield
        # q col-packed: pass i accumulates et=2i into rows 0:64 and et=2i+1
        # into rows 64:128; the two partials are summed on DVE and the
        # result duplicated into both halves of qq.
        psQ = pao.tile([128, 4, 128], F32, tag="a", name=f"ppsQ{b}_{r}")
        for i in range(4):
            xo = xt_own[:, (2 * i) * HALF + c0:(2 * i) * HALF + c1]
            xo2 = xt_own[:, (2 * i + 1) * HALF + c0:(2 * i + 1) * HALF + c1]
            nc.tensor.matmul(psQ[0:64, :, :], wqp2[:, i, 0:64], xo,
                             start=(i == 0), stop=(i == 3))
            nc.tensor.matmul(psQ[64:128, :, :], wqp2[:, i, 64:128], xo2,
                             start=(i == 0), stop=(i == 3))
            yield
        nc.vector.tensor_copy(qq[64:128, c0:c1], psQ[0:64, :, :])
        nc.vector.tensor_add(qq[64:128, c0:c1], qq[64:128, c0:c1],
                             psQ[64:128, :, :])
        nc.vector.tensor_scalar_add(qq[64:128, c0:c1], qq[64:128, c0:c1],
                                    b0q[64:128])
        nc.vector.tensor_copy(qq[0:64, c0:c1], qq[64:128, c0:c1])
        yield

    def vtrans_into(dst, src):
        # One [128,128] transpose covers v tiles (block0,cg) and (block1,cg)
        # -> exchange-payload slots 2cg / 2cg+1, which PV reads directly.
        for cg in range(4):
            pst = ptp.tile([128, 128], DT, tag="tr", name=f"pvt{cg}_{r}")
            nc.tensor.transpose(
                pst, src[:, cg * 128:(cg + 1) * 128], ident_r)
            nc.vector.tensor_copy(
                dst[:, 512 + cg * 128:512 + (cg + 1) * 128], pst)
            yield

    yield from vtrans_into(ex_sb, vtt)
    if local:
        # pd: project the peer half locally with [k|v] weights instead of
        # exchanging — no collective, +16 matmul passes.
        wkv2, bkk = consts["wkv2"], consts["bkk"]
        ex_peer = sb.tile([128, 1024], DT, tag="expeer", name=f"pexpeer_{r}",
                          bufs=PIPE + 1)
        st["ex_peer"] = ex_peer
        vtt2 = sb.tile([128, 512], DT, tag="vtt2", name=f"pvtt2_{r}", bufs=2)
        for b in range(2):
            c0, c1 = b * 512, (b + 1) * 512
            psC = pao.tile([128, 4, 128], F32, tag="a", name=f"ppsC{b}_{r}")
            for et in range(ET):
                xo = xt_oth[:, et * HALF + c0:et * HALF + c1]
                nc.tensor.matmul(psC, wkv2[:, et], xo,
                                 start=(et == 0), stop=(et == ET - 1))
                if et % 2 == 1:
                    yield
            # k from psum rows 0:64 (cross-copy for block 1), v^T staged
            nc.vector.tensor_scalar_add(
                ex_peer[b * 64:(b + 1) * 64, 0:512], psC[0:64, :, :], bkk[0:64])
            nc.vector.tensor_copy(
                vtt2[b * 64:(b + 1) * 64, :], psC[64:128, :, :])
            yield
        yield from vtrans_into(ex_peer, vtt2)
        return
    if st.get("nc_only"):  # timing-diagnostic variant: no exchange
        return
    ex_local = dr.tile([128, 1024], DT, tag="exl", name=f"pexl_{r}",
                        bufs=PIPE + 2)
    nc.sync.dma_start(out=ex_local[:], in_=ex_sb)
    if st.get("ag"):
        ex_pair = dr.tile([2, 128, 1024], DT, tag="exp", name=f"pexp_{r}",
                          bufs=PIPE + 2)
        nc.gpsimd.collective_compute(
            "AllGather", mybir.AluOpType.bypass,
            replica_groups=[[0, 1], [2, 3], [4, 5], [6, 7]],
            ins=[ex_local.opt()], outs=[ex_pair.opt()],
        )
        st["ex_pair"] = ex_pair
        return
    ex_sum = dr.tile([128, 1024], DT, tag="exs", name=f"pexs_{r}",
                      bufs=PIPE + 2)
    nc.gpsimd.collective_compute(
        "AllReduce", mybir.AluOpType.add,
        replica_groups=[[0, 1], [2, 3], [4, 5], [6, 7]],
        ins=[ex_local.opt()], outs=[ex_sum.opt()],
    )
    st["ex_sum"] = ex_sum


def _attn_gen_p(nc, tc, DT, dram, consts, r, st, pools):
    """Pipelined-cc attention of rep r (peer k/v recovery, v transposes,
    row-packed score matmuls, exp, PV accumulation, normalization)."""
    out = dram["out"]
    ex_sb, qq = st["ex_sb"], st["qq"]
    ident, ident_r, bvb = consts["ident"], consts["ident_r"], consts["bvb"]
    sb, psS, psO = pools["sb"], pools["psS"], pools["psO"]
    ptp, ppool, opool = pools["ptp"], pools["pp"], pools["op"]

    # Slot stride 80 when fp8: DoubleRow requires the Ko-dim byte step to be
    # a multiple of 16.
    VDT, VW = (FP8, 80) if PV_FP8 else (DT, H + 1)
    vsb = sb.tile([128, 16, VW], VDT, tag="vsb", name=f"pvsb_{r}", bufs=2)
    ones = vsb[:, :, H:H + 1]
    nc.vector.memset(
        ones.bitcast(F32) if DT == mybir.dt.float32r else ones, 1.0)
    f_all = sb.tile([128, 8, H], F32, tag="fa", name=f"pfa_{r}", bufs=2)
    if st.get("ag"):
        # Both halves arrive via the AllGather; slot order = rank order,
        # which only permutes key order (softmax is order-invariant).
        exg = sb.tile([128, 2, 1024], DT, tag="exg", name=f"pexg_{r}", bufs=2)
        for sl in range(2):
            nc.gpsimd.dma_start(out=exg[:, sl, :], in_=st["ex_pair"][sl])
        ex_sb = exg[:, 0, :]
        ex_peer = exg[:, 1, :]
    elif st.get("local"):
        ex_peer = st["ex_peer"]
    else:
        ex_peer = sb.tile([128, 1024], DT, tag="expeer", name=f"pexpeer_{r}",
                          bufs=2)

    # v arrives pre-transposed in the exchange payload: plain copies into
    # the ones-column-augmented vsb slots.
    nc.vector.tensor_copy(vsb[:, 0:8, 0:H], ex_sb[:, 512:1024])
    yield
    # Recover the peer's [k|v] tile: peer = pair-sum - mine. The
    # AllReduce was launched two pipeline blocks ago, so this is a
    # cheap semaphore check, not a stall.
    if st.get("local") or st.get("ag"):
        pass  # ex_peer came from local projection or the AllGather
    elif st.get("nc_only"):  # diagnostic: fake the peer half (wrong results)
        nc.vector.tensor_copy(ex_peer, ex_sb)
    else:
        ex_sum_sb = sb.tile([128, 1024], DT, tag="exsum", name=f"pexsum_{r}",
                            bufs=2)
        nc.gpsimd.dma_start(out=ex_sum_sb, in_=st["ex_sum"][:])
        nc.vector.tensor_sub(ex_peer, ex_sum_sb, ex_sb)
    nc.vector.tensor_copy(vsb[:, 8:16, 0:H], ex_peer[:, 512:1024])
    yield

    def phase_c(qb, sub, o_ps):
        o_t = opool.tile([H + 1, 128], F32, tag="ot", name=f"pot{qb}{sub}_{r}")
        nc.vector.tensor_copy(o_t, o_ps[:, sub * 128:(sub + 1) * 128])
        ps2 = ptp.tile([128, H + 1], F32, tag="tr", name=f"pc{qb}{sub}_{r}")
        nc.tensor.transpose(ps2, o_t, ident[0:H + 1, 0:H + 1])
        r_t = opool.tile([128, 1], F32, tag="rt", name=f"prt{qb}{sub}_{r}")
        nc.vector.reciprocal(r_t, ps2[:, H:H + 1])
        fa = f_all[:, qb * 4 + sub, :]
        nc.vector.tensor_scalar_mul(fa, ps2[:, 0:H], r_t)
        nc.vector.tensor_add(fa, fa, bvb)

    # Flat group pipeline with one group of S/exp lookahead so the PE never
    # sits in program order behind an exp it must wait for: emission order is
    # S(0), [S(g+1), PV(g)] for g=0..15.
    groups = [(qb, pr) for qb in range(2) for pr in range(8)]
    o_pss, p_tiles = {}, {}

    def emit_s(gi):
        qb, pr = groups[gi]
        if pr == 0:
            o_pss[qb] = psO.tile([H + 1, 512], F32, tag="o", name=f"po{qb}_{r}")
        src = ex_sb if pr < 4 else ex_peer
        prl = pr % 4
        s_ps = psS.tile([128, 2, 512], F32, tag="s", name=f"ps{gi}_{r}")
        for j in range(2):
            lhsT = src[j * 64:(j + 1) * 64, prl * 128:(prl + 1) * 128]
            rhs = qq[j * 64:(j + 1) * 64, qb * 512:(qb + 1) * 512]
            nc.tensor.matmul(s_ps[:, j], lhsT, rhs, start=True, stop=True)
        p_sb = ppool.tile([128, 2, 512], FP8 if PV_FP8 else DT, tag="p",
                          name=f"pp{gi}_{r}")
        nc.scalar.activation(
            p_sb, s_ps, mybir.ActivationFunctionType.Exp, scale=SCALE)
        p_tiles[gi] = p_sb

    pending = []
    emit_s(0)
    for gi in range(16):
        qb, pr = groups[gi]
        if gi + 1 < 16:
            emit_s(gi + 1)
        prl = pr % 4
        p_sb = p_tiles.pop(gi)
        if PV_FP8:
            # One DoubleRow matmul contracts both key-tiles of the pair:
            # lhsT [Ki=128, Ko=2, 65], rhs [Ki=128, Ko=2, 512].
            base = prl * 2 + (8 if pr >= 4 else 0)
            nc.tensor.matmul(
                o_pss[qb], vsb[:, base:base + 2, 0:H + 1], p_sb,
                perf_mode=mybir.MatmulPerfMode.DoubleRow,
                start=(pr == 0), stop=(pr == 7))
        else:
            for j in range(2):
                s_id = prl * 2 + j + (8 if pr >= 4 else 0)
                nc.tensor.matmul(
                    o_pss[qb], vsb[:, s_id, :], p_sb[:, j],
                    start=(pr == 0 and j == 0), stop=(pr == 7 and j == 1))
        if pr == 7:
            pending.extend((qb, sub, o_pss[qb]) for sub in range(4))
        elif pending:
            phase_c(*pending.pop(0))
        yield
    while pending:
        phase_c(*pending.pop(0))
        yield
    nc.sync.dma_start(
        out=out.rearrange("(t p) d -> p t d", p=128), in_=f_all)


def _emit_p(nc, tc, DT, dram, consts, reps, nc_only=False, local=False,
            ag=False):
    """Software-pipelined emission: block i interleaves attention of rep i-2
    with the front (DMA/proj/exchange-launch) of rep i. All tiles come from
    shared ring-buffered pools so cross-rep reuse is WAR-tracked, not
    stack-scoped."""
    with ExitStack() as ctx:
        pools = dict(
            sb=ctx.enter_context(tc.tile_pool(name="pSB", bufs=3)),
            dr=ctx.enter_context(tc.tile_pool(name="pDR", bufs=4, space="DRAM")),
            pao=ctx.enter_context(tc.tile_pool(name="pPA", bufs=1, space="PSUM")),
            psS=ctx.enter_context(tc.tile_pool(name="pss", bufs=2, space="PSUM")),
            psO=ctx.enter_context(tc.tile_pool(name="pso", bufs=2, space="PSUM")),
            ptp=ctx.enter_context(tc.tile_pool(name="ptp", bufs=1, space="PSUM")),
            pp=ctx.enter_context(tc.tile_pool(name="ppb", bufs=3)),
            op=ctx.enter_context(tc.tile_pool(name="pob", bufs=4)),
        )
        sts = [dict(nc_only=nc_only, local=local, ag=ag) for _ in range(reps)]
        fronts = {}

        def start_front(j):
            # Creating the gen + first next() allocates rep j's tiles and
            # issues its x DMAs — done one block early so the data is
            # resident before the interleaved projection matmuls need it.
            if 0 <= j < reps and j not in fronts:
                fronts[j] = _front_gen_p(nc, tc, DT, dram, consts, j, sts[j],
                                         pools)
                next(fronts[j], None)

        for i in range(reps + PIPE):
            start_front(i)
            start_front(i + 1)
            fg = fronts.get(i)
            a = i - PIPE
            if 0 <= a < reps:
                for _ in _attn_gen_p(nc, tc, DT, dram, consts, a, sts[a],
                                     pools):
                    # Pull 2 front steps per attention yield so the front
                    # (29 steps vs 22 yields) exhausts mid-attention: its
                    # tail (v transposes + exchange launch) then interleaves
                    # between S/PV groups instead of bursting serially at
                    # the block boundary, and the collective launches early.
                    if fg is not None:
                        next(fg, None)
                        next(fg, None)
            if fg is not None:
                for _ in fg:
                    pass


def build(mm_mode: str = "f32r", reps: int = 1, upto: str = "full", layout: str = "dup"):
    """Builds + compiles the SPMD single-core program. Returns nc."""
    DT = {"f32r": mybir.dt.float32r, "f32": F32,
          "bf16": mybir.dt.bfloat16}[mm_mode]

    nc = bacc.Bacc("TRN2", target_bir_lowering=False)

    dram = {
        "xt": nc.dram_tensor("xt", [2, 128, 4, ET * HALF // 4], DT, kind="ExternalInput"),
        "wvq": nc.dram_tensor("wvq", [128, ET, 128], DT, kind="ExternalInput"),
        "wk": nc.dram_tensor("wk", [128, ET, 128], DT, kind="ExternalInput"),
        "wvk": nc.dram_tensor("wvk", [128, ET, 128], DT, kind="ExternalInput"),
        "b0q": nc.dram_tensor("b0q", [128, 1], F32, kind="ExternalInput"),
        "b0k": nc.dram_tensor("b0k", [128, 1], F32, kind="ExternalInput"),
        "bvb": nc.dram_tensor("bvb", [128, H], F32, kind="ExternalInput"),
        "wkv2": nc.dram_tensor("wkv2", [128, ET, 128], DT, kind="ExternalInput"),
        "wq": nc.dram_tensor("wq", [128, ET, H], DT, kind="ExternalInput"),
        "bkv": nc.dram_tensor("bkv", [128, 1], F32, kind="ExternalInput"),
        "bq64": nc.dram_tensor("bq64", [H, 1], F32, kind="ExternalInput"),
        "bkk": nc.dram_tensor("bkk", [128, 1], F32, kind="ExternalInput"),
        "wqp2": nc.dram_tensor("wqp2", [128, 4, 128], DT, kind="ExternalInput"),
        "out": nc.dram_tensor("out", [HALF, H], F32, kind="ExternalOutput"),
    }

    with tile.TileContext(nc) as tc, ExitStack() as ctx:
        cp = ctx.enter_context(tc.tile_pool(name="consts", bufs=1))
        consts = {}
        wvq_sb = cp.tile([128, ET, 128], DT, tag="wvq")
        nc.sync.dma_start(out=wvq_sb, in_=dram["wvq"][:])
        wk_sb = cp.tile([128, ET, 128], DT, tag="wk")
        nc.sync.dma_start(out=wk_sb, in_=dram["wk"][:])
        wvk_sb = cp.tile([128, ET, 128], DT, tag="wvk")
        nc.sync.dma_start(out=wvk_sb, in_=dram["wvk"][:])
        b0q_sb = cp.tile([128, 1], F32, tag="b0q")
        nc.sync.dma_start(out=b0q_sb, in_=dram["b0q"][:])
        b0k_sb = cp.tile([128, 1], F32, tag="b0k")
        nc.sync.dma_start(out=b0k_sb, in_=dram["b0k"][:])
        bvb_sb = cp.tile([128, H], F32, tag="bvb")
        nc.sync.dma_start(out=bvb_sb, in_=dram["bvb"][:])
        ident = cp.tile([128, 128], F32, tag="ident")
        make_identity(nc, ident)
        ident_r = cp.tile([128, 128], DT, tag="ident_r")
        nc.vector.tensor_copy(ident_r, ident)
        if layout in ("cc", "cc2"):
            wkv2_sb = cp.tile([128, ET, 128], DT, tag="wkv2")
            nc.sync.dma_start(out=wkv2_sb, in_=dram["wkv2"][:])
            wq_sb = cp.tile([128, ET, H], DT, tag="wq")
            nc.sync.dma_start(out=wq_sb, in_=dram["wq"][:])
            bkv_sb = cp.tile([128, 1], F32, tag="bkv")
            nc.sync.dma_start(out=bkv_sb, in_=dram["bkv"][:])
            bq64_sb = cp.tile([H, 1], F32, tag="bq64")
            nc.sync.dma_start(out=bq64_sb, in_=dram["bq64"][:])
            consts.update(wkv2=wkv2_sb, wq=wq_sb, bkv=bkv_sb, bq64=bq64_sb)
        if layout in ("p", "pnc", "pd", "pag"):
            bkk_sb = cp.tile([128, 1], F32, tag="bkk")
            nc.sync.dma_start(out=bkk_sb, in_=dram["bkk"][:])
            wkv2_sb = cp.tile([128, ET, 128], DT, tag="wkv2")
            nc.sync.dma_start(out=wkv2_sb, in_=dram["wkv2"][:])
            wqp2_sb = cp.tile([128, 4, 128], DT, tag="wqp2")
            nc.sync.dma_start(out=wqp2_sb, in_=dram["wqp2"][:])
            consts.update(wkk=wk_sb, bkk=bkk_sb, wkv2=wkv2_sb, wqp2=wqp2_sb)
        consts.update(
            wvq=wvq_sb, wk=wk_sb, wvk=wvk_sb, b0q=b0q_sb, b0k=b0k_sb,
            bvb=bvb_sb, ident=ident, ident_r=ident_r,
        )

        if layout in ("p", "pnc", "pd", "pag"):
            _emit_p(nc, tc, DT, dram, consts, reps,
                    nc_only=(layout == "pnc"), local=(layout == "pd"),
                    ag=(layout == "pag"))
        else:
            for r in range(reps):
                _emit_rep(nc, tc, DT, dram, consts, r, upto, layout)

    nc.compile()
    return nc


def shard_inputs(x, Wq, bq, Wk, bk, Wv, bv, mm_mode="f32r"):
    """Builds the per-core input maps (host-side layout prep)."""
    if mm_mode == "bf16":
        import ml_dtypes
        dt_np = ml_dtypes.bfloat16
    else:
        dt_np = np.float32
    x = np.asarray(x, dtype=np.float32).astype(dt_np)
    Wq, Wk, Wv = (np.asarray(a, np.float32).astype(dt_np) for a in (Wq, Wk, Wv))
    bq, bk, bv = (np.asarray(a, np.float32) for a in (bq, bk, bv))
    z = np.zeros(64, np.float32)
    zw = np.zeros((E, 0), dtype=dt_np)

    def eperm(w):  # [E, d] -> [128, ET, d] with row (p, t) = w[8p + t]
        return np.ascontiguousarray(w.reshape(128, ET, -1))

    wvq = eperm(np.concatenate([Wv, Wq], axis=1))
    wvk = eperm(np.concatenate([Wv, Wk], axis=1))
    wkv2 = eperm(np.concatenate([Wk, Wv], axis=1))
    wqp = eperm(Wq)
    # [128, 4, 128]: pass i holds [Wq(et=2i) | Wq(et=2i+1)] for the
    # col-packed q projection (psum rows 0:64 = even-et partials, 64:128 odd)
    wqp2 = np.ascontiguousarray(
        np.concatenate([wqp[:, 0::2, :], wqp[:, 1::2, :]], axis=2))
    bkv = np.concatenate([bk, z])[:, None].copy()
    bkk2 = np.concatenate([bk, bk])[:, None].copy()
    bq64 = bq[:, None].copy()
    b0q = np.concatenate([z, bq])[:, None].copy()
    b0k = np.concatenate([z, bk])[:, None].copy()
    bvb = np.ascontiguousarray(np.broadcast_to(bv, (128, H)))
    in_maps = []
    for c in range(NCORES):
        b, h = divmod(c, 2)
        own = x[b, h * HALF:(h + 1) * HALF].T        # [E, 1024]
        oth = x[b, (1 - h) * HALF:(2 - h) * HALF].T  # [E, 1024]
        # [2, 128, 4, 2048]: (half, p, chunk, j): e-row 8p + (chunk*2 + j//1024)
        xt = np.stack([own, oth]).reshape(2, 128, 8, HALF)
        xt = np.ascontiguousarray(xt.reshape(2, 128, 4, ET * HALF // 4))
        wkk = eperm(np.concatenate([Wk, Wk], axis=1))
        in_maps.append({
            "xt": xt, "wvq": wvq, "wk": wkk, "wvk": wvk,
            "b0q": b0q, "b0k": b0k, "bvb": bvb,
            "wkv2": wkv2, "wq": wqp, "bkv": bkv, "bq64": bq64,
            "bkk": bkk2, "wqp2": wqp2,
        })
    return in_maps


def gather_outputs(results):
    out = np.empty((B, S, H), np.float32)
    for c in range(NCORES):
        b, h = divmod(c, 2)
        out[b, h * HALF:(h + 1) * HALF] = results[c]["out"]
    return out


_NC_CACHE = {}


def _get_nc(mm_mode="f32r", reps=1, upto="full", layout="dup"):
    key = (mm_mode, reps, upto, layout)
    if key not in _NC_CACHE:
        _NC_CACHE[key] = build(mm_mode, reps, upto, layout)
    return _NC_CACHE[key]


def run(inputs, mm_mode="f32r", layout="cc", **kw):
    from concourse.bass_utils import run_bass_kernel_spmd

    nc = _get_nc(mm_mode, layout=layout)
    in_maps = shard_inputs(**inputs, mm_mode=mm_mode)
    res = run_bass_kernel_spmd(nc, in_maps, core_ids=list(range(NCORES)), **kw)
    return gather_outputs(res.results), res


def _build_exec(nc, in_maps):
    """Builds a re-invokable (non-donating) sharded executable + device args.

    Mirrors bass2jax.run_bass_via_pjrt's multi-core path, but keeps the
    output buffers as ordinary (non-donated) device arrays so the same
    callable can be executed repeatedly for wall-clock timing.
    """
    import jax
    from jax.sharding import Mesh, PartitionSpec, NamedSharding
    from jax.experimental.shard_map import shard_map
    from concourse import mybir
    from concourse.bass2jax import (
        _bass_exec_p, partition_id_tensor, install_neuronx_cc_hook,
    )

    install_neuronx_cc_hook()
    partition_name = nc.partition_id_tensor.name if nc.partition_id_tensor else None
    in_names, out_names, out_avals, zero_outs = [], [], [], []
    for alloc in nc.m.functions[0].allocations:
        if not isinstance(alloc, mybir.MemoryLocationSet):
            continue
        name = alloc.memorylocations[0].name
        if alloc.kind == "ExternalInput":
            if name != partition_name:
                in_names.append(name)
        elif alloc.kind == "ExternalOutput":
            out_names.append(name)
            shape = tuple(alloc.tensor_shape)
            dtype = mybir.dt.np(alloc.dtype)
            out_avals.append(jax.core.ShapedArray(shape, dtype))
            zero_outs.append(np.zeros(shape, dtype))
    n_params = len(in_names)
    all_in_names = list(in_names) + list(out_names)
    if partition_name is not None:
        all_in_names.append(partition_name)

    def _body(*args):
        operands = list(args)
        if partition_name is not None:
            operands.append(partition_id_tensor())
        outs = _bass_exec_p.bind(
            *operands,
            out_avals=tuple(out_avals),
            in_names=tuple(all_in_names),
            out_names=tuple(out_names),
            lowering_input_output_aliases=(),
            sim_require_finite=True,
            sim_require_nnan=True,
            nc=nc,
        )
        return tuple(outs)

    n_cores = len(in_maps)
    devices = jax.devices()[:n_cores]
    mesh = Mesh(np.asarray(devices), ("core",))
    nin = n_params + len(out_names)
    sharded = jax.jit(
        shard_map(
            _body, mesh=mesh,
            in_specs=(PartitionSpec("core"),) * nin,
            out_specs=(PartitionSpec("core"),) * len(out_names),
            check_rep=False,
        ),
        keep_unused=True,
    )
    sh = NamedSharding(mesh, PartitionSpec("core"))
    dev_args = [
        jax.device_put(
            np.concatenate([np.asarray(m[i]) for m in in_maps], axis=0), sh
        )
        for i in in_names
    ] + [
        jax.device_put(
            np.zeros((n_cores * z.shape[0], *z.shape[1:]), z.dtype), sh
        )
        for z in zero_outs
    ]
    return sharded, dev_args, out_names, out_avals


def _exec_results(r, out_names, out_avals):
    out_arrs = [np.asarray(a) for a in r]
    return [
        {
            name: out_arrs[i].reshape(NCORES, *out_avals[i].shape)[c]
            for i, name in enumerate(out_names)
        }
        for c in range(NCORES)
    ]


def bench(inputs, mm_mode="f32r", iters=50, reps=1, upto="full", layout="dup",
          n_cores=NCORES):
    """Amortized wall-clock per-execution time over repeated runs."""
    import jax, time

    nc = _get_nc(mm_mode, reps, upto, layout)
    in_maps = shard_inputs(**inputs, mm_mode=mm_mode)[:n_cores]
    fn, dev_args, out_names, out_avals = _build_exec(nc, in_maps)
    r = fn(*dev_args)
    jax.block_until_ready(r)  # compile + warm
    t0 = time.perf_counter()
    for _ in range(iters):
        r = fn(*dev_args)
    jax.block_until_ready(r)
    dt = (time.perf_counter() - t0) / iters
    if n_cores != NCORES:
        return None, dt
    return gather_outputs(_exec_results(r, out_names, out_avals)), dt


def kernel(**inputs) -> np.ndarray:
    # Retry the primary layout: the first collective execution after another
    # process released the devices can transiently desync the mesh; a brief
    # pause lets it recover before the next attempt.
    import time as _time

    attempts = (("bf16", "p"), ("bf16", "p"), ("bf16", "p"),
                ("f32r", "cc"), ("f32r", "dup"))
    for mm_mode, layout in attempts:
        try:
            out, _ = run(inputs, mm_mode=mm_mode, layout=layout)
            return out
        except Exception:
            _time.sleep(2.0)
            continue
    raise RuntimeError("all kernel layouts failed")

